# revision 43
# baseline (speedup 1.0000x reference)
"""GIN (4-layer) message-passing kernel for 8 trn2 NeuronCores (Bass/Tile), v3.

Key structure (vs the straightforward per-layer GIN):
- Tables exchanged between layers live in "z-space": t'_v = rz_v @ Wc.T with
  Wc = W1[l+1] @ W2[l] folded on the host. Then z_{l+1} = (I+A) t' directly:
  the one-hot aggregation matmuls produce z with NO separate mm1 and no
  y0 intermediate. b1 is exactly absorbed by BatchNorm (constant per-feature
  shift); b2 is added during the mm2' PSUM evacuation.
- fp8 tables/one-hots with DoubleRow matmuls (2 contraction chunks/pass).
- Nodes sharded by dst range across 8 cores; per layer one fp8 table
  AllGather split in two halves (A: local rows < HL, B: rest) so the A-half
  AG fires when half of mm2' is done and early windows' A-gathers overlap
  the B-half AG flight.  Tiny dummy AR+AG at program start absorb the
  first-collective warmup / core-alignment cost.
- Edges grouped per 128-dst window, split by src-table half, padded to
  KA/KB 128-edge chunks; gathered via dma_gather over 4 SWDGE queues
  (7 windows per call).
- BN stats in one DVE bn_stats pass per 512-block + bn_aggr, converted to
  [sum, sumsq] lanes, [128, 8] fp32 AllReduce, BN+ReLU applied in-place in
  z on ACT (two node-range halves so early mm2' windows unblock sooner),
  accum_out rsums feed the outputs.
- Outputs (node-means of last two layers) via linearity on the host:
  mean(rz @ W2.T + b2) = (colsum rz)/N @ W2.T + b2.
"""

import os
import numpy as np
import ml_dtypes

N_NODES = 50000
N_EDGES = 150000
D = 512
L = 4
NC = 8
BN_EPS = 1e-5
P = 128
W = 128          # dst window size
WPC = 7          # windows per gather call
FCH = D // P     # feature chunks (4)

TSCALE = 8.0     # scale of fp8 tables (t0 and t')
F8 = ml_dtypes.float8_e4m3   # IEEE e4m3 (max 240) == TRN float8e4
WDT_BF16 = True   # wct in bf16 (fp8 weight path hangs this runtime)
KNODR = bool(os.environ.get("KNODR", ""))      # DoubleRow implicated in device hangs here
KNOSUMS = bool(os.environ.get("KNOSUMS"))  # Sum(z) on DVE instead of PE (debug)
KNOB2 = bool(os.environ.get("KNOB2"))     # skip b2 row DMA (debug)
KNOIP = bool(os.environ.get("KNOIP"))     # avoid in-place z add (debug)
KSQDVE = bool(os.environ.get("KSQDVE"))   # z^2 on DVE instead of ACT (debug)
KNOEARLY = bool(os.environ.get("KNOEARLY"))  # no early AG_A fire (debug)
KNOCOLL = bool(os.environ.get("KNOCOLL"))    # collectives -> local DMA (timing diag)
KNOAR = bool(os.environ.get("KNOAR"))        # AllReduce -> local DMA (timing diag)
KNOAG = bool(os.environ.get("KNOAG"))        # AllGather -> local DMA (timing diag)
KWARM = not bool(os.environ.get("KNOWARM"))  # dummy warm-up collectives at start
KFUSE = not bool(os.environ.get("KSPLIT"))   # fused single-pass aggregation
KPSB = int(os.environ.get("KPSB", "6"))      # aggA psum bufs (fused frees aggB+sums banks)
KMMB = int(os.environ.get("KMMB", "2"))      # mm2 psum bufs
KGB = int(os.environ.get("KGB", "3"))        # gather tile bufs
KSP = bool(os.environ.get("KSP"))            # single_packet gathers (A/B)
KQS = bool(os.environ.get("KQS"))            # spread gathers over all 4 queues

_CACHE = {}
LAST_EXEC_NS = None
_LAST_RUN = None  # (nc, in_maps) for timing helpers


# --------------------------------------------------------------- host prep
def _plan(src, dst, n_nodes):
    """Edge planning: per-core gather indices + one-hot M_T blocks.

    Table A holds rows i < HL of every core (rowA = r*HL + i); table B
    holds rows i >= HL (rowB = r*HL + (i-HL)) so each table stays under the
    int16 gather-index limit and the two AllGather halves can fire at
    different times.  Edges are grouped per (dst window, table half) into
    KA/KB padded 128-edge chunks.
    """
    NL = n_nodes // NC
    HL = NL // 2
    NW = (NL + W - 1) // W

    src = np.asarray(src).astype(np.int64)
    dst = np.asarray(dst).astype(np.int64)

    core = dst // NL
    dloc = dst % NL
    win = dloc // W
    r = src // NL
    i = src % NL
    half = (i >= HL).astype(np.int64)
    trow = np.where(half == 0, r * HL + i, r * HL + (i - HL))
    drel = dloc - win * W

    # group = ((c*NW + w)*2 + half); edges sorted by group (trow within)
    group = (core * NW + win) * 2 + half
    order = np.lexsort((trow, group))
    group, trow, drel = group[order], trow[order], drel[order]
    ngrp = NC * NW * 2
    cnt = np.bincount(group, minlength=ngrp)            # edges per group
    cntA = cnt[0::2].reshape(NC, NW)
    cntB = cnt[1::2].reshape(NC, NW)
    start = np.zeros(ngrp, np.int64)
    np.cumsum(cnt[:-1], out=start[1:])
    rank = np.arange(len(group)) - start[group]         # slot within group

    KA = max(1, -(-int(cntA.max()) // P))
    KB = max(1, -(-int(cntB.max()) // P))
    KC = KA + KB
    NCALL = -(-NW // WPC)
    NIDXA, NIDXB = WPC * KA * P, WPC * KB * P

    idxA = np.zeros((NC, NCALL * NIDXA), np.int64)
    idxB = np.zeros((NC, NCALL * NIDXB), np.int64)
    # flat idx position for each edge
    c_e = group // (2 * NW)
    w_e = (group // 2) % NW
    h_e = group % 2
    ca_e, wo_e = w_e // WPC, w_e % WPC
    posA = ca_e * NIDXA + wo_e * KA * P + rank
    posB = ca_e * NIDXB + wo_e * KB * P + rank
    mA = h_e == 0
    idxA[c_e[mA], posA[mA]] = trow[mA]
    idxB[c_e[~mA], posB[~mA]] = trow[~mA]


    # one-hot mt: [NC, NW, KC*P, W] (A chunks then B chunks)
    mt = np.zeros((NC, NW, KC * P, W), np.float32)
    slot = np.where(mA, rank, KA * P + rank)
    flat = slot + (c_e * NW + w_e) * (KC * P)
    np.add.at(mt.reshape(-1), flat * W + drel, 1.0)

    WS = W
    mt2 = mt

    def wrap(ix, nidx):  # per call: [NIDX] -> [128, NIDX//16] wrapped+replicated
        ncall = ix.shape[-1] // nidx
        blk = ix.reshape(NC, ncall, nidx // 16, 16)
        out = np.transpose(blk, (0, 3, 1, 2)).reshape(NC, 16, ncall * (nidx // 16))
        return np.tile(out, (1, 8, 1)).astype(np.int16)

    # pack mt into per-4-window groups [NSW, P, 4*KC*WS] fp8
    NSW = -(-NW // 4)
    mtg = np.zeros((NC, NSW, P, 4 * KC * WS), F8)
    mtr = mt2.reshape(NC, NW, KC, P, WS)
    for g in range(NSW):
        n = min(4, NW - 4 * g)
        # [NC, n, KC, P, WS] -> [NC, P, n, KC, WS]
        mtg[:, g, :, : n * KC * WS] = np.transpose(
            mtr[:, 4 * g : 4 * g + n], (0, 3, 1, 2, 4)).reshape(NC, P, n * KC * WS)
    assert idxA.max(initial=0) < NC * HL <= 32768
    assert idxB.max(initial=0) < NC * HL <= 32768
    return dict(NL=NL, HL=HL, NW=NW, KA=KA, KB=KB, KC=KC, NCALL=NCALL,
                NIDXA=NIDXA, NIDXB=NIDXB,
                idxA=wrap(idxA, NIDXA), idxB=wrap(idxB, NIDXB), mt=mtg)


def _q8(a):
    return np.clip(a, -224.0, 224.0).astype(F8)


def _prep_weights(W1, gamma, beta, W2, b2, layers):
    """wct[l] = packed TSCALE * (W1[l+1] @ W2[l]).T per feature chunk,
    b2p[l] = TSCALE * W1[l+1] @ b2[l]; packs = gamma/beta columns + eps."""
    wdt = ml_dtypes.bfloat16 if WDT_BF16 else F8
    wct = np.zeros((layers - 1, P, FCH * D), wdt)
    b2p = np.zeros((layers - 1, P, D), np.float32)
    for li in range(layers - 1):
        Wc = (W1[li + 1].astype(np.float64) @ W2[li].astype(np.float64))
        Wq = TSCALE * Wc   # t' = rz @ (S*Wc).T
        for f in range(FCH):
            blk = Wq[:, f * P : (f + 1) * P].T.astype(np.float32)  # [P(d), D(j)]
            wct[li, :, f * D : (f + 1) * D] = _q8(blk) if not WDT_BF16 else blk.astype(ml_dtypes.bfloat16)
        b2p[li] = np.tile((TSCALE * (W1[li + 1].astype(np.float64)
            @ b2[li].astype(np.float64))).astype(np.float32)[None, :], (P, 1))
    packs = np.zeros((P, 2 * layers * FCH + 1), np.float32)
    packs[:, -1] = BN_EPS
    for li in range(layers):
        for j in range(FCH):
            packs[:, li * FCH + j] = gamma[li, j * P : (j + 1) * P]
            packs[:, layers * FCH + li * FCH + j] = beta[li, j * P : (j + 1) * P]
    return wct, b2p, packs


# --------------------------------------------------------------- builder
def _build(cfg, layers, n_nodes, reps=1):
    import concourse.bass as bass
    import concourse.bacc as bacc
    import concourse.tile as tile
    from concourse import mybir

    bf16, f32, i16 = mybir.dt.bfloat16, mybir.dt.float32, mybir.dt.int16
    f8 = mybir.dt.float8e4
    wdt = bf16 if WDT_BF16 else f8
    NL, HL, NW = cfg["NL"], cfg["HL"], cfg["NW"]
    KA, KB, KC = cfg["KA"], cfg["KB"], cfg["KC"]
    NCALL, NIDXA, NIDXB = cfg["NCALL"], cfg["NIDXA"], cfg["NIDXB"]
    NHA, NHB = NC * HL, NC * HL
    SA, SB = NIDXA // 16, NIDXB // 16
    NSW = -(-NW // 4)
    NPAD = NW * P
    NB = -(-NL // 512)
    WMID = (NW + 1) // 2                   # windows covering rows [0, HL)
    DR = mybir.MatmulPerfMode.DoubleRow
    ACTW = 11                              # t'-evac windows on ACT

    nc = bacc.Bacc("TRN2", target_bir_lowering=False, debug=False,
                   num_devices=NC, num_swdge_queues=4)

    xA_t = nc.dram_tensor("xA", [NHA, D], f8, kind="ExternalInput")
    xB_t = nc.dram_tensor("xB", [NHB, D], f8, kind="ExternalInput")
    t0own_t = nc.dram_tensor("t0own", [NL, D], f8, kind="ExternalInput")
    idxA_t = nc.dram_tensor("idxA", [P, NCALL * SA], i16, kind="ExternalInput")
    idxB_t = nc.dram_tensor("idxB", [P, NCALL * SB], i16, kind="ExternalInput")
    WS = W
    mt_t = nc.dram_tensor("mt", [NSW, P, 4 * KC * WS], f8, kind="ExternalInput")
    wct_t = nc.dram_tensor("wct", [layers - 1, P, FCH * D], wdt, kind="ExternalInput")
    b2p_t = nc.dram_tensor("b2p", [layers - 1, P, D], f32, kind="ExternalInput")
    packs_t = nc.dram_tensor("packs", [P, 2 * layers * FCH + 1], f32, kind="ExternalInput")
    ident_t = nc.dram_tensor("ident", [P, P], f8, kind="ExternalInput")
    out_t = nc.dram_tensor("out", [2, P, FCH], f32, kind="ExternalOutput")

    rg = [list(range(NC))]
    Relu = mybir.ActivationFunctionType.Relu
    Sqrt = mybir.ActivationFunctionType.Sqrt
    Square = mybir.ActivationFunctionType.Square
    Alu = mybir.AluOpType
    AX = mybir.AxisListType.X

    with tile.TileContext(nc) as tc:
        with (
            tc.tile_pool(name="persist", bufs=1) as pers,
            tc.tile_pool(name="ga", bufs=KGB) as gap,
            tc.tile_pool(name="gb", bufs=KGB) as gbp,
            tc.tile_pool(name="zp", bufs=1) as zpool,
            tc.tile_pool(name="hqp", bufs=2) as hqp,
            tc.tile_pool(name="wp", bufs=1) as wp,
            tc.tile_pool(name="sm", bufs=1) as sm,
            tc.tile_pool(name="aggAps", bufs=KPSB if KFUSE else 3,
                         space="PSUM") as aggAps,
            tc.tile_pool(name="aggBps", bufs=2, space="PSUM") as aggBps,
            tc.tile_pool(name="mm2ps", bufs=KMMB if KFUSE else 2,
                         space="PSUM") as mm2ps,
            tc.tile_pool(name="dram", bufs=1, space="DRAM") as dr,
        ):
            idxA_sb = pers.tile([P, NCALL * SA], i16)
            nc.sync.dma_start(out=idxA_sb[:], in_=idxA_t[:])
            idxB_sb = pers.tile([P, NCALL * SB], i16)
            nc.sync.dma_start(out=idxB_sb[:], in_=idxB_t[:])
            ident_sb = pers.tile([P, P], f8)
            nc.sync.dma_start(out=ident_sb[:], in_=ident_t[:])
            packs_sb = pers.tile([P, 2 * layers * FCH + 1], f32)
            nc.sync.dma_start(out=packs_sb[:], in_=packs_t[:])
            mt_sb = pers.tile([P, NSW * 4 * KC * WS], f8)
            nc.sync.dma_start(
                out=mt_sb[:].rearrange("p (g c) -> p g c", g=NSW),
                in_=mt_t[:].rearrange("g p c -> p g c"))
            wct_sb = pers.tile([P, (layers - 1) * FCH * D], wdt)
            nc.sync.dma_start(
                out=wct_sb[:].rearrange("p (l c) -> p l c", l=layers - 1),
                in_=wct_t[:].rearrange("l p c -> p l c"))
            outsb = pers.tile([P, 2 * FCH], f32)
            nc.vector.memset(outsb[:], 0.0)

            if KWARM and not (KNOCOLL or KNOAR or KNOAG):
                # Tiny dummy AR+AG: pays the first-collective warmup /
                # core-alignment cost concurrently with layer-0 compute
                # (which needs no collectives).
                wri = dr.tile([P, 4], f32, name="warm_ri", tag="warm_ri")
                wro = dr.tile([P, 4], f32, addr_space="Shared",
                              name="warm_ro", tag="warm_ro")
                nc.sync.dma_start(out=wri[:], in_=packs_t[:, 0:4])
                nc.gpsimd.collective_compute(
                    "AllReduce", Alu.add, replica_groups=rg,
                    ins=[wri.opt()], outs=[wro.opt()])
                wgi = dr.tile([P, D], f8, name="warm_gi", tag="warm_gi")
                wgo = dr.tile([NC * P, D], f8, addr_space="Shared",
                              name="warm_go", tag="warm_go")
                nc.sync.dma_start(out=wgi[0:P, 0:P], in_=ident_t[:])
                nc.sync.dma_start(out=wgi[0:P, P : 2 * P], in_=ident_t[:])
                nc.sync.dma_start(out=wgi[0:P, 2 * P : 3 * P], in_=ident_t[:])
                nc.sync.dma_start(out=wgi[0:P, 3 * P : 4 * P], in_=ident_t[:])
                nc.gpsimd.collective_compute(
                    "AllGather", Alu.bypass, replica_groups=rg,
                    ins=[wgi.opt()], outs=[wgo.opt()])

            def pack_ap(sect, l, j):
                o = sect * layers * FCH + l * FCH + j
                return packs_sb[:, o : o + 1]

            def mm2x(out, lhsT2, rhs2, start, stop, skip=False):
                """DoubleRow pair (or two plain matmuls under KNODR)."""
                if KNODR:
                    nc.tensor.matmul(out=out, lhsT=lhsT2[:, 0], rhs=rhs2[:, 0],
                                     start=start, stop=False,
                                     skip_group_check=skip)
                    nc.tensor.matmul(out=out, lhsT=lhsT2[:, 1], rhs=rhs2[:, 1],
                                     start=False, stop=stop,
                                     skip_group_check=skip)
                else:
                    nc.tensor.matmul(out=out, lhsT=lhsT2, rhs=rhs2,
                                     start=start, stop=stop, perf_mode=DR,
                                     skip_group_check=skip)

            for rl in range(reps * layers):
                rep, l = divmod(rl, layers)
                TL = NL - (NW - 1) * P          # valid rows in last window
                TLa = (TL // 32) * 32           # 32-aligned partition start
                if l == 0:
                    tabA, tabB = xA_t, xB_t
                    hq_prev = hqp.tile([P, NW, D], f8,
                                       name=f"hq_init{rep}", tag="hq")
                    if TL < P:
                        nc.vector.memset(hq_prev[TLa:P, NW - 1, :], 0.0)
                    nwf0 = NL // P
                    nc.sync.dma_start(
                        out=hq_prev[:, 0:nwf0, :],
                        in_=t0own_t[0 : nwf0 * P, :].rearrange(
                            "(t p) d -> p t d", p=P))
                    if NL % P:
                        nc.sync.dma_start(
                            out=hq_prev[0 : NL - nwf0 * P, nwf0, :],
                            in_=t0own_t[nwf0 * P : NL, :])
                last = l == layers - 1

                # ---------- gather + aggregation (single phase) -> z --------
                z = zpool.tile([P, FCH, NPAD], bf16, name=f"z_{rl}", tag="z")
                ga_t = gb_t = None
                # -- pass A: self-term + A-table chunks -> z (copy evac).
                #    Needs only AG_A of the previous layer, so it runs while
                #    AG_B is still in flight.  (KFUSE: B-chunks fused here,
                #    no pass B.)
                for w in range(NW):
                    ca, wo = w // WPC, w % WPC
                    if wo == 0:
                        ga_t = gap.tile([P, WPC * KA, D], f8,
                                        name=f"ga{rl}_{ca}", tag="ga")
                        nc.gpsimd.dma_gather(
                            out_ap=ga_t[:], in_ap=tabA[:],
                            idxs_ap=idxA_sb[:, ca * SA : (ca + 1) * SA],
                            num_idxs=NIDXA, num_idxs_reg=NIDXA, elem_size=D,
                            single_packet=KSP,
                            queue_num=(ca % 4) if KQS else (2 * ca) % 4,
                        )
                        if KFUSE:
                            gb_t = gbp.tile([P, WPC * KB, D], f8,
                                            name=f"gb{rl}_{ca}", tag="gb")
                            nc.gpsimd.dma_gather(
                                out_ap=gb_t[:], in_ap=tabB[:],
                                idxs_ap=idxB_sb[:, ca * SB : (ca + 1) * SB],
                                num_idxs=NIDXB, num_idxs_reg=NIDXB, elem_size=D,
                                single_packet=KSP,
                            queue_num=((ca + 2) % 4) if KQS else (2 * ca + 1) % 4,
                            )
                    mtw = mt_sb[:, (w // 4) * (4 * KC * WS) + (w % 4) * KC * WS :
                                ].rearrange("p (k c) -> p k c", c=WS)
                    psum_w = aggAps.tile([P, FCH * W], f32, name=f"agA{rl}_{w}",
                                         tag="agg", space="PSUM")
                    for f in range(FCH):
                        first = True
                        for k in range(0, KA - 1, 2):
                            mm2x(psum_w[:, f * W : (f + 1) * W],
                                 ga_t[:, wo * KA + k : wo * KA + k + 2,
                                      f * P : (f + 1) * P],
                                 mtw[:, k : k + 2, 0:W],
                                 start=first, stop=False)
                            first = False
                        if KA % 2:
                            k = KA - 1
                            nc.tensor.matmul(
                                out=psum_w[:, f * W : (f + 1) * W],
                                lhsT=ga_t[:, wo * KA + k, f * P : (f + 1) * P],
                                rhs=mtw[:, k, 0:W], start=first, stop=False)
                            first = False
                        if KFUSE:
                            for k in range(0, KB - 1, 2):
                                mm2x(psum_w[:, f * W : (f + 1) * W],
                                     gb_t[:, wo * KB + k : wo * KB + k + 2,
                                          f * P : (f + 1) * P],
                                     mtw[:, KA + k : KA + k + 2, 0:W],
                                     start=first, stop=False)
                                first = False
                            if KB % 2:
                                k = KB - 1
                                nc.tensor.matmul(
                                    out=psum_w[:, f * W : (f + 1) * W],
                                    lhsT=gb_t[:, wo * KB + k,
                                              f * P : (f + 1) * P],
                                    rhs=mtw[:, KA + k, 0:W],
                                    start=first, stop=False)
                                first = False
                        nc.tensor.matmul(
                            out=psum_w[:, f * W : (f + 1) * W],
                            lhsT=hq_prev[:, w, f * P : (f + 1) * P],
                            rhs=ident_sb[:],
                            start=first, stop=True,
                        )
                    nc.scalar.copy(
                        out=z[:, :, w * P : (w + 1) * P],
                        in_=psum_w[:].rearrange("p (f d) -> p f d", f=FCH),
                    )

                # -- pass B: B-table chunks -> psum, added into z (DVE).
                for w in ([] if KFUSE else range(NW)):
                    ca, wo = w // WPC, w % WPC
                    if wo == 0:
                        gb_t = gbp.tile([P, WPC * KB, D], f8,
                                        name=f"gb{rl}_{ca}", tag="gb")
                        nc.gpsimd.dma_gather(
                            out_ap=gb_t[:], in_ap=tabB[:],
                            idxs_ap=idxB_sb[:, ca * SB : (ca + 1) * SB],
                            num_idxs=NIDXB, num_idxs_reg=NIDXB, elem_size=D,
                            single_packet=KSP,
                            queue_num=((ca + 2) % 4) if KQS else (2 * ca + 1) % 4,
                        )
                    mtw = mt_sb[:, (w // 4) * (4 * KC * WS) + (w % 4) * KC * WS :
                                ].rearrange("p (k c) -> p k c", c=WS)
                    psum_b = aggBps.tile([P, FCH * W], f32, name=f"agB{rl}_{w}",
                                         tag="aggb", space="PSUM")
                    for f in range(FCH):
                        first = True
                        for k in range(0, KB - 1, 2):
                            mm2x(psum_b[:, f * W : (f + 1) * W],
                                 gb_t[:, wo * KB + k : wo * KB + k + 2,
                                      f * P : (f + 1) * P],
                                 mtw[:, KA + k : KA + k + 2, 0:W],
                                 start=first, stop=(k == KB - 2 and not KB % 2))
                            first = False
                        if KB % 2:
                            k = KB - 1
                            nc.tensor.matmul(
                                out=psum_b[:, f * W : (f + 1) * W],
                                lhsT=gb_t[:, wo * KB + k, f * P : (f + 1) * P],
                                rhs=mtw[:, KA + k, 0:W], start=first, stop=True)
                    nc.vector.tensor_tensor(
                        out=z[:, :, w * P : (w + 1) * P],
                        in0=psum_b[:].rearrange("p (f d) -> p f d", f=FCH),
                        in1=z[:, :, w * P : (w + 1) * P], op=Alu.add,
                    )

                # ---------------- BN stats (DVE, one bn_stats pass) --------
                bnst = sm.tile([P, FCH * NB * 6], f32, name=f"bst{rl}", tag="bnst")
                mv = sm.tile([P, 2 * FCH], f32, name=f"mv{rl}", tag="mv")
                stat = sm.tile([P, 2 * FCH], f32, name=f"st{rl}", tag="stat")
                for f in range(FCH):
                    for nb in range(NB):
                        n0 = nb * 512
                        nw512 = min(512, NL - n0)
                        o6 = (f * NB + nb) * 6
                        nc.vector.bn_stats(
                            out=bnst[:, o6 : o6 + 6],
                            in_=z[:, f, n0 : n0 + nw512])
                for f in range(FCH):
                    nc.vector.bn_aggr(
                        out=mv[:, 2 * f : 2 * f + 2],
                        in_=bnst[:, f * NB * 6 : (f + 1) * NB * 6])
                # stat = [sum, sumsq]: sum = NL*mean; sumsq = NL*(var + mean^2)
                mvr = mv[:].rearrange("p (f two) -> p f two", two=2)
                means = mvr[:, :, 0:1].rearrange("p f one -> p (f one)")
                vars_ = mvr[:, :, 1:2].rearrange("p f one -> p (f one)")
                sum_o, ssq_o = stat[:, 0:FCH], stat[:, FCH : 2 * FCH]
                nc.vector.tensor_tensor(out=ssq_o, in0=means, in1=means,
                                        op=Alu.mult)
                nc.vector.tensor_tensor(out=ssq_o, in0=vars_, in1=ssq_o,
                                        op=Alu.add)
                nc.vector.tensor_copy(out=sum_o, in_=means)
                nc.vector.tensor_scalar_mul(stat[:], stat[:], float(NL))

                # ---------------- BN stats AllReduce ----------------
                ar_in = dr.tile([P, 2 * FCH], f32, name=f"arin{rl}", tag=f"arin{rl}")
                ar_out = dr.tile([P, 2 * FCH], f32, addr_space="Shared",
                                 name=f"arout{rl}", tag=f"arout{rl}")
                nc.sync.dma_start(out=ar_in[:], in_=stat[:])
                if KNOCOLL or KNOAR:
                    nc.sync.dma_start(out=ar_out[:], in_=ar_in[:])
                else:
                    nc.gpsimd.collective_compute(
                        "AllReduce", Alu.add, replica_groups=rg,
                        ins=[ar_in.opt()], outs=[ar_out.opt()],
                    )
                gstat = sm.tile([P, 2 * FCH], f32, name=f"gst{rl}", tag="gstat")
                nc.sync.dma_start(out=gstat[:], in_=ar_out[:])

                # mu/var -> scale s, bias t  (all [P, FCH] lanes)
                inv_n = 1.0 / float(n_nodes)
                bnp = sm.tile([P, 6 * FCH], f32, name=f"bnp{rl}", tag="bnp")
                MU, EX2, VAR, SD, SS, TT = range(6)

                def bs(k):
                    return bnp[:, k * FCH : (k + 1) * FCH]

                nc.vector.tensor_scalar_mul(bs(MU), gstat[:, 0:FCH], inv_n)
                nc.vector.tensor_scalar_mul(bs(EX2), gstat[:, FCH : 2 * FCH], inv_n)
                nc.vector.tensor_tensor(out=bs(VAR), in0=bs(MU), in1=bs(MU),
                                        op=Alu.mult)
                nc.vector.tensor_tensor(out=bs(VAR), in0=bs(EX2), in1=bs(VAR),
                                        op=Alu.subtract)
                nc.scalar.activation(out=bs(SD), in_=bs(VAR), func=Sqrt,
                                     bias=packs_sb[:, -1:])
                nc.vector.reciprocal(out=bs(SS), in_=bs(SD))
                gam = packs_sb[:, l * FCH : (l + 1) * FCH]
                bet = packs_sb[:, (layers + l) * FCH : (layers + l + 1) * FCH]
                nc.vector.tensor_tensor(out=bs(SS), in0=bs(SS), in1=gam,
                                        op=Alu.mult)
                nc.vector.tensor_tensor(out=bs(TT), in0=bs(MU), in1=bs(SS),
                                        op=Alu.mult)
                nc.vector.tensor_tensor(out=bs(TT), in0=bet, in1=bs(TT),
                                        op=Alu.subtract)

                # ------------- BN apply + ReLU -> rz (in-place in z) -------
                # Two halves so mm2' windows [0, WMID) unblock after the
                # first-half applies (AG_A then fires earlier).
                rz = z
                H1 = min(WMID * P, NL)
                rsum = sm.tile([P, 2 * FCH], f32, name=f"rsum{rl}", tag="rsum")
                for h, (h0, h1) in enumerate(((0, H1), (H1, NL))):
                    for j in range(FCH):
                        nc.scalar.activation(
                            out=rz[:, j, h0:h1], in_=z[:, j, h0:h1], func=Relu,
                            bias=bnp[:, TT * FCH + j : TT * FCH + j + 1],
                            scale=bnp[:, SS * FCH + j : SS * FCH + j + 1],
                            accum_out=rsum[:, h * FCH + j : h * FCH + j + 1],
                        )
                if l >= layers - 2:
                    row = 0 if last else 1
                    nc.vector.tensor_tensor(
                        out=outsb[:, row * FCH : (row + 1) * FCH],
                        in0=rsum[:, 0:FCH], in1=rsum[:, FCH : 2 * FCH],
                        op=Alu.add)

                # ---------------- mm2' -> t' ; bounce ; AllGather -----------
                if not last:
                    hq = hqp.tile([P, NW, D], f8, name=f"hq_{rl}", tag="hq")
                    if TL < P:
                        nc.vector.memset(hq[TLa:P, NW - 1, :], 0.0)
                    b2b_sb = wp.tile([P, D], f32, name=f"b2b{rl}", tag="b2b")
                    nc.sync.dma_start(out=b2b_sb[:], in_=b2p_t[l])
                    wct_l = wct_sb[:, l * FCH * D : (l + 1) * FCH * D].rearrange(
                        "p (f d) -> p f d", f=FCH)
                    for w in range(NW):
                        n0 = w * P
                        nw = min(P, NL - n0)
                        hp = mm2ps.tile([P, D], f32, name=f"hp{rl}_{w}",
                                        tag="mm2", space="PSUM")
                        if WDT_BF16:
                            for k in range(FCH):
                                nc.tensor.matmul(
                                    out=hp[:nw, :],
                                    lhsT=rz[:, k, n0 : n0 + nw],
                                    rhs=wct_l[:, k, :],
                                    start=(k == 0), stop=(k == FCH - 1),
                                )
                        else:
                            for k in range(0, FCH - 1, 2):
                                mm2x(hp[:nw, :],
                                     rz[:, k : k + 2, n0 : n0 + nw],
                                     wct_l[:, k : k + 2, :],
                                     start=(k == 0), stop=(k == FCH - 2))
                        nc.vector.tensor_tensor(
                            out=hq[:nw, w, :], in0=hp[:nw, :],
                            in1=b2b_sb[:nw, :], op=Alu.add)

                    hbA = dr.tile([HL, D], f8, name=f"hbA{rl}", tag=f"hbA{rl}")
                    hbB = dr.tile([HL, D], f8, name=f"hbB{rl}", tag=f"hbB{rl}")
                    hfA = dr.tile([NHA, D], f8, addr_space="Shared",
                                  name=f"hfA{rl}", tag=f"hfA{rl}")
                    hfB = dr.tile([NHB, D], f8, addr_space="Shared",
                                  name=f"hfB{rl}", tag=f"hfB{rl}")
                    nwf = HL // P
                    soff = HL % P
                    nc.sync.dma_start(
                        out=hbA[0 : nwf * P, :].rearrange("(t p) d -> p t d", p=P),
                        in_=hq[:, 0:nwf, :])
                    if soff:
                        nc.sync.dma_start(out=hbA[nwf * P : HL, :],
                                          in_=hq[0:soff, nwf, :])
                        nc.sync.dma_start(out=hbB[0 : P - soff, :],
                                          in_=hq[soff:P, nwf, :])
                    o = (P - soff) if soff else 0
                    nwf2 = (NL - (nwf + 1) * P) // P
                    if nwf2 > 0:
                        nc.sync.dma_start(
                            out=hbB[o : o + nwf2 * P, :].rearrange(
                                "(t p) d -> p t d", p=P),
                            in_=hq[:, nwf + 1 : nwf + 1 + nwf2, :])
                    o2 = o + nwf2 * P
                    if o2 < HL:
                        nc.sync.dma_start(
                            out=hbB[o2:HL, :],
                            in_=hq[0 : HL - o2, nwf + 1 + nwf2, :])
                    if KNOCOLL or KNOAG:
                        nc.sync.dma_start(out=hfA[0:HL, :], in_=hbA[:])
                        nc.sync.dma_start(out=hfB[0:HL, :], in_=hbB[:])
                    else:
                        nc.gpsimd.collective_compute(
                            "AllGather", Alu.bypass, replica_groups=rg,
                            ins=[hbA.opt()], outs=[hfA.opt()],
                        )
                        nc.gpsimd.collective_compute(
                            "AllGather", Alu.bypass, replica_groups=rg,
                            ins=[hbB.opt()], outs=[hfB.opt()],
                        )
                    tabA, tabB = hfA, hfB
                    hq_prev = hq
            nc.sync.dma_start(
                out=out_t[:].rearrange("r p f -> p r f"),
                in_=outsb[:].rearrange("p (r f) -> p r f", r=2))

    nc.compile()
    return nc


# --------------------------------------------------------------- entry
def kernel(x, W1, b1, gamma, beta, W2, b2, src, dst):
    from concourse.bass_utils import run_bass_kernel_spmd

    n_nodes = x.shape[0]
    layers = W1.shape[0]
    plan = _plan(src, dst, n_nodes)
    NL, HL = plan["NL"], plan["HL"]

    wct, b2p, packs = _prep_weights(
        np.asarray(W1, np.float32), np.asarray(gamma, np.float32),
        np.asarray(beta, np.float32), np.asarray(W2, np.float32),
        np.asarray(b2, np.float32), layers)

    # layer-0 table: t0 = TSCALE * x @ W1[0].T (b1 absorbed by BN)
    xf = np.asarray(x, np.float32)
    t0 = _q8(TSCALE * (xf @ np.asarray(W1[0], np.float32).T))
    t3 = t0.reshape(NC, NL, D)
    xA = np.ascontiguousarray(t3[:, :HL, :])
    xB = np.ascontiguousarray(t3[:, HL:, :])
    ident = np.eye(P, dtype=F8)

    key = (n_nodes, layers, plan["KA"], plan["KB"], plan["NW"], WDT_BF16, WPC,
           KNODR, KNOSUMS, KNOB2, KNOIP, KSQDVE, KNOEARLY, KNOCOLL,
           KNOAR, KNOAG, KWARM, KFUSE, KPSB, KMMB, KGB, KSP, KQS)
    if key not in _CACHE:
        _CACHE[key] = _build(plan, layers, n_nodes)
    nc = _CACHE[key]

    in_maps = []
    for c in range(NC):
        in_maps.append({
            "xA": xA.reshape(-1, D), "xB": xB.reshape(-1, D),
            "t0own": np.ascontiguousarray(t3[c]),
            "idxA": plan["idxA"][c], "idxB": plan["idxB"][c],
            "mt": plan["mt"][c],
            "wct": wct, "b2p": b2p, "packs": packs, "ident": ident,
        })
    global _LAST_RUN
    _LAST_RUN = (nc, in_maps)
    res = run_bass_kernel_spmd(nc, in_maps, core_ids=list(range(NC)))

    rsum = np.zeros((2, P, FCH), np.float64)
    for c in range(NC):
        rsum += res.results[c]["out"].astype(np.float64)
    W2f = np.asarray(W2, np.float64)
    b2f = np.asarray(b2, np.float64)
    outs = []
    for row, li in ((0, layers - 1), (1, layers - 2)):
        colsum = rsum[row].T.reshape(D)
        outs.append(((colsum @ W2f[li].T) / n_nodes + b2f[li]).astype(np.float32))
    return (outs[0], outs[1])


# --------------------------------------------------------------- timing
def measure_exec_ns(reps=12):
    """Estimate on-device exec time per kernel invocation.

    The axon tunnel adds a large, roughly constant round-trip to every
    synchronous dispatch; we time the kernel executable and a tiny
    calibration executable the same way and subtract.  (NTFF profiling is
    unavailable through this tunnel, and chaining custom_calls inside one
    executable is rejected by the neuronx compile hook.)
    """
    import time
    import numpy as np
    import jax
    from jax.sharding import Mesh, PartitionSpec, NamedSharding
    try:
        from jax.experimental.shard_map import shard_map
    except ImportError:
        from jax.sharding import shard_map
    from concourse import bass2jax
    import concourse.mybir as mybir

    assert _LAST_RUN is not None, "call kernel() first"
    nc, in_maps = _LAST_RUN
    bass2jax.install_neuronx_cc_hook()

    partition_name = nc.partition_id_tensor.name if nc.partition_id_tensor else None
    in_names, out_names, out_avals, zero_outs = [], [], [], []
    for alloc in nc.m.functions[0].allocations:
        if not isinstance(alloc, mybir.MemoryLocationSet):
            continue
        name = alloc.memorylocations[0].name
        if alloc.kind == "ExternalInput":
            if name != partition_name:
                in_names.append(name)
        elif alloc.kind == "ExternalOutput":
            shape = tuple(alloc.tensor_shape)
            dtype = mybir.dt.np(alloc.dtype)
            out_names.append(name)
            out_avals.append(jax.core.ShapedArray(shape, dtype))
            zero_outs.append(np.zeros(shape, dtype))
    n_params = len(in_names)
    n_outs = len(out_avals)
    all_in_names = list(in_names) + list(out_names)
    if partition_name is not None:
        all_in_names.append(partition_name)

    def _body(*args):
        operands = list(args)
        if partition_name is not None:
            operands.append(bass2jax.partition_id_tensor())
        outs = bass2jax._bass_exec_p.bind(
            *operands,
            out_avals=tuple(out_avals),
            in_names=tuple(all_in_names),
            out_names=tuple(out_names),
            lowering_input_output_aliases=(),
            sim_require_finite=True,
            sim_require_nnan=True,
            nc=nc,
        )
        return tuple(outs)

    devices = jax.devices()[:NC]
    mesh = Mesh(np.asarray(devices), ("core",))
    spec = PartitionSpec("core")
    sharding = NamedSharding(mesh, spec)
    n_in = n_params + n_outs
    fn = jax.jit(shard_map(
        _body, mesh=mesh, in_specs=(spec,) * n_in,
        out_specs=(spec,) * n_outs, check_rep=False))

    concat_in = [
        jax.device_put(
            np.concatenate([np.asarray(in_maps[c][nm]) for c in range(NC)], axis=0),
            sharding)
        for nm in in_names
    ]
    concat_zeros = [
        jax.device_put(np.zeros((NC * z.shape[0], *z.shape[1:]), z.dtype), sharding)
        for z in zero_outs
    ]

    tiny_in = jax.device_put(np.zeros((NC, 128), np.float32), sharding)
    tiny = jax.jit(shard_map(lambda a: a + 1.0, mesh=mesh,
                             in_specs=(spec,), out_specs=spec, check_rep=False))

    def sync_min(f, args, n):
        jax.block_until_ready(f(*args))        # warm
        best = float("inf")
        for _ in range(n):
            t0 = time.perf_counter()
            jax.block_until_ready(f(*args))
            best = min(best, time.perf_counter() - t0)
        return best

    t_k = sync_min(fn, (*concat_in, *concat_zeros), reps)
    t_0 = sync_min(tiny, (tiny_in,), reps)
    ns = max(0.0, (t_k - t_0)) * 1e9
    global LAST_EXEC_NS
    LAST_EXEC_NS = int(ns)
    return LAST_EXEC_NS

